# revision 3
# baseline (speedup 1.0000x reference)
"""Trainium2 Bass kernel for ChebyshevAdditiveAngularMargin loss.

Reference computation (per element of a [N, C] f32 matrix):
    cosine = clip(outputs, -1+eps, 1-eps)
    phi    = clenshaw(cosine, coeffs)            # degree-30 Chebyshev
    phi    = where(cosine > TH, phi, cosine - MM)
    out    = SCALE * (targets * phi + (1 - targets) * cosine)

`targets` is a one-hot matrix (one 1.0 per row), so out == SCALE*cosine
everywhere except a single element per row.  The kernel therefore:
  1. extracts the hot cosine per row exactly via a fused multiply-reduce
     (sum of targets*x: all non-hot products are exactly 0.0),
  2. runs the exact 31-step Clenshaw recurrence on just [128, 1]
     per-row values (matching jax's fp32 op order),
  3. scatters the correction back with one fused op:
     out = (targets * delta[row]) + cosine, then * SCALE.

Rows are sharded across 8 NeuronCores (data parallel); the coefficient
vector is baked into the instruction stream as immediates.
"""

import sys

sys.path.insert(0, "/opt/trn_rl_repo")

import numpy as np

import concourse.bacc as bacc
import concourse.mybir as mybir
from concourse.tile import TileContext

F32 = mybir.dt.float32
OP = mybir.AluOpType

N, C = 8192, 8192
N_CORES = 8
ROWS = N // N_CORES  # rows per core
P = 128  # SBUF partitions

MARGIN = 0.2
SCALE = 30.0
EPS = 1e-07
TH = float(np.cos(np.pi - MARGIN))
MM = float(np.sin(np.pi - MARGIN) * MARGIN)
CLIP_LO = -1.0 + EPS
CLIP_HI = 1.0 - EPS


def build_bass(rows: int, cols: int, coeffs: np.ndarray):
    """Build the per-core program. Each core processes [rows, cols]."""
    cs = [float(c) for c in coeffs]  # f32 values, baked as immediates
    deg = len(cs) - 1
    n_blocks = rows // P

    nc = bacc.Bacc("TRN2", target_bir_lowering=False)
    x_d = nc.dram_tensor("outputs", [rows, cols], F32, kind="ExternalInput")
    t_d = nc.dram_tensor("targets", [rows, cols], F32, kind="ExternalInput")
    o_d = nc.dram_tensor("out", [rows, cols], F32, kind="ExternalOutput")

    with TileContext(nc) as tc:
        with (
            tc.tile_pool(name="xp", bufs=2) as xp,
            tc.tile_pool(name="tp", bufs=2) as tp,
            tc.tile_pool(name="scratch", bufs=1) as sp,
            tc.tile_pool(name="tiny", bufs=2) as yp,
        ):
            scratch = sp.tile([P, cols], F32)  # ttr mandatory full-size out
            for b in range(n_blocks):
                xt = xp.tile([P, cols], F32, tag="xt")
                tt = tp.tile([P, cols], F32, tag="tt")
                nc.sync.dma_start(xt[:], x_d[b * P : (b + 1) * P, :])
                nc.sync.dma_start(tt[:], t_d[b * P : (b + 1) * P, :])

                # --- extraction: s_raw[p] = sum_c targets*x (== hot x, exact)
                s_raw = yp.tile([P, 1], F32, tag="s_raw")
                nc.vector.scalar_tensor_tensor(
                    scratch[:], tt[:], 1.0, xt[:], OP.mult, OP.mult,
                    accum_out=s_raw[:],
                )

                # --- big pass A: cosine = clip(x), in place
                nc.vector.tensor_scalar(
                    xt[:], xt[:], CLIP_HI, CLIP_LO, OP.min, OP.max
                )

                # --- tiny path: clip, Clenshaw (faithful fp32 op order),
                #     threshold-select, delta
                s = yp.tile([P, 1], F32, tag="s")
                x2s = yp.tile([P, 1], F32, tag="x2s")
                nc.vector.tensor_scalar(
                    s[:], s_raw[:], CLIP_HI, CLIP_LO, OP.min, OP.max
                )
                nc.vector.tensor_scalar_mul(x2s[:], s[:], 2.0)

                b1 = yp.tile([P, 1], F32, tag="b1")
                b2 = yp.tile([P, 1], F32, tag="b2")
                bn = yp.tile([P, 1], F32, tag="bn")
                tm = yp.tile([P, 1], F32, tag="tm")
                nc.vector.memset(b1[:], cs[deg])  # step k=deg from (0,0)
                nc.vector.memset(b2[:], 0.0)
                for k in range(deg - 1, -1, -1):
                    # b_new = (c_k + x2*b1) - b2, rounded exactly like jax:
                    # t = fl(x2*b1); b_new = fl(fl(t + c_k) - b2)
                    nc.vector.tensor_tensor(tm[:], x2s[:], b1[:], OP.mult)
                    nc.vector.scalar_tensor_tensor(
                        bn[:], tm[:], cs[k], b2[:], OP.add, OP.subtract
                    )
                    b1, b2, bn = bn, b1, b2
                # phi = b0 - b1*x  (b0 = b1 after rotation, b1 = b2)
                nc.vector.tensor_tensor(tm[:], b2[:], s[:], OP.mult)
                phi = yp.tile([P, 1], F32, tag="phi")
                nc.vector.tensor_tensor(phi[:], b1[:], tm[:], OP.subtract)

                # phisel = where(s > TH, phi, s - MM)
                mask = yp.tile([P, 1], F32, tag="mask")
                alt = yp.tile([P, 1], F32, tag="alt")
                diff = yp.tile([P, 1], F32, tag="diff")
                nc.vector.tensor_scalar(mask[:], s[:], TH, None, OP.is_gt)
                nc.vector.tensor_scalar_sub(alt[:], s[:], MM)
                nc.vector.tensor_tensor(diff[:], phi[:], alt[:], OP.subtract)
                phisel = yp.tile([P, 1], F32, tag="phisel")
                nc.vector.scalar_tensor_tensor(
                    phisel[:], diff[:], mask[:], alt[:], OP.mult, OP.add
                )
                delta = yp.tile([P, 1], F32, tag="delta")
                nc.vector.tensor_tensor(delta[:], phisel[:], s[:], OP.subtract)

                # --- big pass C: out = (targets * delta[row]) + cosine
                nc.vector.scalar_tensor_tensor(
                    xt[:], tt[:], delta[:], xt[:], OP.mult, OP.add
                )
                # --- big pass D: out *= SCALE
                nc.vector.tensor_scalar_mul(xt[:], xt[:], SCALE)

                nc.sync.dma_start(o_d[b * P : (b + 1) * P, :], xt[:])
    return nc


_TRACE = False  # test.py sets this to capture an NTFF profile
_LAST_RESULTS = None


def kernel(outputs: np.ndarray, targets: np.ndarray, coeffs: np.ndarray) -> np.ndarray:
    global _LAST_RESULTS
    from concourse.bass_utils import run_bass_kernel_spmd

    assert outputs.shape == (N, C) and targets.shape == (N, C)
    nc = build_bass(ROWS, C, np.asarray(coeffs))
    nc.finalize()
    in_maps = [
        {
            "outputs": np.ascontiguousarray(outputs[i * ROWS : (i + 1) * ROWS]),
            "targets": np.ascontiguousarray(targets[i * ROWS : (i + 1) * ROWS]),
        }
        for i in range(N_CORES)
    ]
    res = run_bass_kernel_spmd(
        nc, in_maps, core_ids=list(range(N_CORES)), trace=_TRACE
    )
    _LAST_RESULTS = res
    return np.concatenate([r["out"] for r in res.results], axis=0)
